# revision 14
# baseline (speedup 1.0000x reference)
"""CrossAttention Trainium2 kernel (8-core SPMD).

Sharding: core c = (b, g) with b = c // 2 (batch), g = c % 2 (head group of 8).
Each core computes the full attention + partial output projection for its
(batch, 8-head group); the host sums the two partial o-proj results per batch.

Per-core device pipeline (all matmuls fp32r, N=512):
  1. PE-transpose x[b], enc[b] -> xT, eT (C on partitions).
  2. Projections in natural layout: Q,K (T part, 8h x 64d free), V likewise;
     l2-norm (free-dim reduce) + partial rotary applied in natural layout.
  3. PE-transpose Q,K -> qT,kT (head-dims on partitions, T free).
  4. scoresT[k,q] = K @ Q^T accumulated in PSUM with PE-transposed bias tiles;
     exp on ACT; causal masking via memset + triangular-mask multiply;
     AV via lhsT = [V | ones] giving y^T and softmax denominators in one pass.
  5. Normalize y^T by the broadcast reciprocal denominator; o-proj from the
     head-pair-stacked y^T; DMA partial (T, C) result out.
"""

import os
import sys
from contextlib import ExitStack

import numpy as np

if not os.path.isdir(os.path.join(os.path.dirname(os.path.abspath(__file__)), "concourse")):
    for _p in ("/opt/trn_rl_repo",):
        if os.path.isdir(_p) and _p not in sys.path:
            sys.path.insert(0, _p)

import concourse.bass as bass  # noqa: E402
import concourse.tile as tile  # noqa: E402
from concourse import bacc, mybir  # noqa: E402
from concourse.bass_utils import run_bass_kernel_spmd  # noqa: E402

B, T, C = 4, 1024, 1024
H, KV, D = 16, 8, 64
L = 32
HG = 8          # heads per group (= kv heads; local head l uses kv head l)
NG = 2          # head groups
QK_NORM_SCALE = 10.0
DS = float(D) ** -0.5
SCALE_Q = DS * DS / QK_NORM_SCALE   # folded into q's rsqrt(norm) factor

F32 = mybir.dt.float32
F32R = mybir.dt.float32r

NT = T // 128   # 8 T-tiles
NC_ = C // 128  # 8 C-tiles


def r(ap):
    return ap.bitcast(F32R)


def build_program():
    nc = bacc.Bacc(
        "TRN2",
        target_bir_lowering=False,
        debug=False,
        enable_asserts=False,
        num_devices=8,
    )

    def din(name, shape):
        return nc.dram_tensor(name, shape, F32, kind="ExternalInput").ap()

    xb = din("xb", (T, C))
    eb = din("eb", (T, C))
    wq = din("wq", (C, HG * D))
    wk = din("wk", (C, KV * D))
    wv = din("wv", (C, KV * D))
    wo = din("wo", (HG * D, C))
    bias = din("bias", (HG, T, T))
    cfq = din("cfq", (T, D))
    seq_ = din("seq", (T, L // 2))
    soq = din("soq", (T, L // 2))
    cfk = din("cfk", (T, D))
    sek = din("sek", (T, L // 2))
    sok = din("sok", (T, L // 2))
    cfv = din("cfv", (T, D))
    sev = din("sev", (T, L // 2))
    sov = din("sov", (T, L // 2))
    identf = din("identf", (128, 128))
    tri = din("tri", (128, 128))
    out_d = nc.dram_tensor("out", (T, C), F32, kind="ExternalOutput").ap()

    with tile.TileContext(nc) as tc, ExitStack() as ctx:
        const = ctx.enter_context(tc.tile_pool(name="const", bufs=1))
        persist = ctx.enter_context(tc.tile_pool(name="persist", bufs=1))

        # ---- constants ----
        ident = const.tile([128, 128], F32, tag="ident")
        nc.sync.dma_start(ident[:], identf)
        trim = const.tile([128, 128], F32, tag="trim")
        nc.sync.dma_start(trim[:], tri)

        # rope constants: (T, n) -> (128, NT, n)
        rope_sb = {}
        for nm, ap_, w in (
            ("cfq", cfq, D), ("seq", seq_, 16), ("soq", soq, 16),
            ("cfk", cfk, D), ("sek", sek, 16), ("sok", sok, 16),
            ("cfv", cfv, D), ("sev", sev, 16), ("sov", sov, 16),
        ):
            t_ = const.tile([128, NT * w], F32, tag=nm, name=nm)
            t3 = t_.rearrange("p (tt d) -> p tt d", tt=NT)
            nc.sync.dma_start(t3, ap_.rearrange("(tt p) d -> p tt d", p=128))
            rope_sb[nm] = t3

        # persistent across attention: wo, qT/kT, va, y_stack
        wo_t = persist.tile([128, 4 * C], F32R, tag="wo", name="wo_t")
        wo_sb = wo_t.rearrange("p (pl c) -> p pl c", pl=4)
        nc.sync.dma_start(wo_sb, r(wo.rearrange("(pl p) c -> p pl c", p=128)))
        qT = [persist.tile([128, T], F32R, tag=f"qT{pl}", name=f"qT{pl}") for pl in range(4)]
        kT = [persist.tile([128, T], F32R, tag=f"kT{pl}", name=f"kT{pl}") for pl in range(4)]
        va = [persist.tile([128, HG * 65], F32R, tag=f"va{tt}", name=f"va{tt}") for tt in range(NT)]
        y_stack = [persist.tile([128, T], F32R, tag=f"ys{pl}", name=f"ys{pl}") for pl in range(4)]

        def rope_inplace(v3, tt, cf, se, so, smallp):
            """v3: (128, HG, d) SBUF view; applies partial rotary in place."""
            ev = v3[:, :, 0:L:2]
            od = v3[:, :, 1:L:2]
            se_b = rope_sb[se][:, tt].unsqueeze(1).broadcast_to([128, HG, 16])
            so_b = rope_sb[so][:, tt].unsqueeze(1).broadcast_to([128, HG, 16])
            cf_b = rope_sb[cf][:, tt].unsqueeze(1).broadcast_to([128, HG, D])
            tmp_e = smallp.tile([128, HG * 16], F32, tag="tmpe", name="tmpe")
            tmp_o = smallp.tile([128, HG * 16], F32, tag="tmpo", name="tmpo")
            te3 = tmp_e.rearrange("p (h d) -> p h d", h=HG)
            to3 = tmp_o.rearrange("p (h d) -> p h d", h=HG)
            nc.gpsimd.tensor_mul(te3, od, se_b)
            nc.gpsimd.tensor_mul(to3, ev, so_b)
            nc.gpsimd.tensor_mul(v3[:, :, 0:D], v3[:, :, 0:D], cf_b)
            nc.gpsimd.tensor_sub(ev, ev, te3)
            nc.gpsimd.tensor_add(od, od, to3)

        def flush_qn(qns, ttg, tpsum, dstT):
            """PE-transpose 4 ready qn tiles into dstT[pl][:, ttg*512:]."""
            for pl in range(4):
                ps4 = tpsum.tile([128, 512], F32, tag="tps", name="tps")
                for tti in range(4):
                    nc.tensor.matmul(
                        ps4[:, tti * 128:(tti + 1) * 128],
                        qns[tti][:, pl * 128:(pl + 1) * 128],
                        ident[:], is_transpose=True, start=True, stop=True,
                    )
                nc.any.tensor_copy(
                    dstT[pl][:, ttg * 512:(ttg + 1) * 512], ps4[:]
                )

        def norm_rope_transpose(ps, tt, which, smallp, sqp, rotp, tpsum, dstT):
            """ps: (128 T, 512) psum of raw projections. Normalizes per head,
            applies rope; returns the qn tile."""
            sq = sqp.tile([128, HG * D], F32, tag="sq", name="sq")
            nc.scalar.square(sq[:], ps[:])
            ss = smallp.tile([128, HG], F32, tag="ss", name="ss")
            nc.vector.tensor_reduce(
                ss[:], sq.rearrange("p (h d) -> p h d", h=HG),
                axis=mybir.AxisListType.X, op=mybir.AluOpType.add,
            )
            inv = smallp.tile([128, HG], F32, tag="inv", name="inv")
            nc.vector.reciprocal(inv[:], ss[:])
            rs = smallp.tile([128, HG], F32, tag="rs", name="rs")
            scl = SCALE_Q * SCALE_Q if which == "q" else 1.0
            nc.scalar.activation(
                rs[:], inv[:], mybir.ActivationFunctionType.Sqrt,
                bias=0.0, scale=scl,
            )
            qn = rotp.tile([128, HG * D], F32, tag="qn", name="qn")
            d3 = qn.rearrange("p (h d) -> p h d", h=HG)
            nc.vector.tensor_mul(
                d3, ps.rearrange("p (h d) -> p h d", h=HG),
                rs[:].unsqueeze(2).broadcast_to([128, HG, D]),
            )
            if which == "q":
                rope_inplace(d3, tt, "cfq", "seq", "soq", smallp)
            else:
                rope_inplace(d3, tt, "cfk", "sek", "sok", smallp)
            return qn

        # ---- x phase: transpose x -> xT, project Q, -> qT ----
        for phase in ("x", "e"):
            with tc.tile_pool(name="srcT", bufs=1) as srcTp, \
                 tc.tile_pool(name="wp", bufs=1) as wp, \
                 tc.tile_pool(name="projp", bufs=3, space="PSUM") as projp, \
                 tc.tile_pool(name="tpsum", bufs=3, space="PSUM") as tpsum, \
                 tc.tile_pool(name="smallp", bufs=4) as smallp, \
                 tc.tile_pool(name="sqp", bufs=2) as sqp, \
                 tc.tile_pool(name="rotp", bufs=5) as rotp:
                srcT = [srcTp.tile([128, T], F32R, tag=f"sT{cb}", name=f"sT{cb}")
                        for cb in range(NC_)]
                with tc.tile_pool(name="natp", bufs=1) as natp:
                    nat = natp.tile([128, NT * C], F32, tag="nat", name="nat")
                    nat3 = nat.rearrange("p (tt c) -> p tt c", tt=NT)
                    nc.sync.dma_start(
                        nat3,
                        (xb if phase == "x" else eb)
                        .rearrange("(tt p) c -> p tt c", p=128),
                    )
                    for cb in range(NC_):
                        for ttg in range(2):
                            ps4 = tpsum.tile([128, 512], F32, tag="tps",
                                             name="tps")
                            for tti in range(4):
                                tt = ttg * 4 + tti
                                nc.tensor.matmul(
                                    ps4[:, tti * 128:(tti + 1) * 128],
                                    nat3[:, tt, cb * 128:(cb + 1) * 128],
                                    ident[:], is_transpose=True,
                                    start=True, stop=True,
                                )
                            nc.any.tensor_copy(
                                srcT[cb][:, ttg * 512:(ttg + 1) * 512], ps4[:]
                            )
                if phase == "x":
                    wq_t = wp.tile([128, NC_ * 512], F32R, tag="wq", name="wq_t")
                    wq_sb = wq_t.rearrange("p (cb n) -> p cb n", cb=NC_)
                    nc.sync.dma_start(
                        wq_sb, r(wq.rearrange("(cb p) n -> p cb n", p=128)))
                    qns = []
                    for tt in range(NT):
                        ps = projp.tile([128, 512], F32, tag="proj", name="proj")
                        for cb in range(NC_):
                            nc.tensor.matmul(
                                ps[:], r(srcT[cb][:, tt * 128:(tt + 1) * 128]),
                                r(wq_sb[:, cb]),
                                start=(cb == 0), stop=(cb == NC_ - 1),
                            )
                        qns.append(norm_rope_transpose(ps, tt, "q", smallp,
                                                       sqp, rotp, tpsum, qT))
                        if tt % 4 == 3:
                            flush_qn(qns[-4:], tt // 4, tpsum, qT)
                else:
                    wk_t = wp.tile([128, NC_ * 512], F32R, tag="wk", name="wk_t")
                    wk_sb = wk_t.rearrange("p (cb n) -> p cb n", cb=NC_)
                    nc.sync.dma_start(
                        wk_sb, r(wk.rearrange("(cb p) n -> p cb n", p=128)))
                    wv_t = wp.tile([128, NC_ * 512], F32R, tag="wv", name="wv_t")
                    wv_sb = wv_t.rearrange("p (cb n) -> p cb n", cb=NC_)
                    nc.sync.dma_start(
                        wv_sb, r(wv.rearrange("(cb p) n -> p cb n", p=128)))
                    kns = []
                    for tt in range(NT):
                        ps = projp.tile([128, 512], F32, tag="proj", name="proj")
                        for cb in range(NC_):
                            nc.tensor.matmul(
                                ps[:], r(srcT[cb][:, tt * 128:(tt + 1) * 128]),
                                r(wk_sb[:, cb]),
                                start=(cb == 0), stop=(cb == NC_ - 1),
                            )
                        kns.append(norm_rope_transpose(ps, tt, "k", smallp,
                                                       sqp, rotp, tpsum, kT))
                        if tt % 4 == 3:
                            flush_qn(kns[-4:], tt // 4, tpsum, kT)
                        # V: no norm; pack into 65-stride with ones column
                        psv = projp.tile([128, 512], F32, tag="proj", name="projv")
                        for cb in range(NC_):
                            nc.tensor.matmul(
                                psv[:], r(srcT[cb][:, tt * 128:(tt + 1) * 128]),
                                r(wv_sb[:, cb]),
                                start=(cb == 0), stop=(cb == NC_ - 1),
                            )
                        v3 = va[tt].rearrange("p (h e) -> p h e", h=HG)
                        nc.vector.tensor_copy(
                            v3[:, :, 0:D],
                            psv.rearrange("p (h d) -> p h d", h=HG),
                        )
                        nc.vector.memset(v3[:, :, D:D + 1].bitcast(F32), 1.0)
                        rope_inplace(v3, tt, "cfv", "sev", "sov", smallp)

        # ---- attention ----
        with tc.tile_pool(name="biasp", bufs=2) as biasp, \
             tc.tile_pool(name="attp", bufs=6) as attp, \
             tc.tile_pool(name="spsum", bufs=4, space="PSUM") as spsum, \
             tc.tile_pool(name="ypsum", bufs=3, space="PSUM") as ypsum, \
             tc.tile_pool(name="smalle", bufs=4) as smalle:
            for lb in range(0, HG, 2):      # head blocks of 2
                btiles = []
                for qt in range(NT):
                    cols = (qt + 1) * 128
                    bt = biasp.tile([128, 2 * cols], F32, tag=f"bias{qt}",
                                    name=f"bias{qt}")
                    bt3 = bt.rearrange("p (h k) -> p h k", h=2)
                    nc.sync.dma_start(
                        bt3,
                        bias[lb:lb + 2, qt * 128:(qt + 1) * 128, 0:cols]
                        .rearrange("h p k -> p h k"),
                    )
                    btiles.append(bt3)
                for l4 in range(2):
                    l = lb + l4
                    pl, sub = l // 2, l % 2
                    po = 64 * sub
                    for qg in range(2):
                        q0 = qg * 512
                        nkt = qg * 4 + 4
                        psy = ypsum.tile([65, 512], F32, tag="psy", name="psy")
                        for kt in range(nkt):
                            pss = spsum.tile([128, 512], F32, tag="pss",
                                             name="pss")
                            nc.tensor.matmul(
                                pss[:],
                                r(kT[pl][po:po + 64, kt * 128:(kt + 1) * 128]),
                                r(qT[pl][po:po + 64, q0:q0 + 512]),
                                start=True, stop=False,
                            )
                            valid_qbs = [qb for qb in range(4)
                                         if kt <= qg * 4 + qb]
                            for j, qb in enumerate(valid_qbs):
                                qt = qg * 4 + qb
                                nc.tensor.matmul(
                                    pss[:, qb * 128:(qb + 1) * 128],
                                    btiles[qt][:, l4, kt * 128:(kt + 1) * 128],
                                    ident[:],
                                    is_transpose=True,
                                    start=False, stop=(j == len(valid_qbs) - 1),
                                )
                            att = attp.tile([128, 512], F32R, tag="att",
                                            name="att")
                            cvf = max(0, kt * 128 - q0)
                            if cvf > 0:
                                nc.vector.memset(att[:, 0:cvf].bitcast(F32), 0.0)
                            nc.scalar.activation(
                                att[:, cvf:512], pss[:, cvf:512],
                                mybir.ActivationFunctionType.Exp,
                            )
                            if kt * 128 >= q0:
                                nc.vector.tensor_mul(
                                    att[:, cvf:cvf + 128],
                                    att[:, cvf:cvf + 128], trim[:],
                                )
                            nc.tensor.matmul(
                                psy[:],
                                r(va[kt][:, l * 65:(l + 1) * 65]),
                                r(att[:]),
                                start=(kt == 0), stop=(kt == nkt - 1),
                            )
                        rcp = smalle.tile([1, 512], F32, tag="rcp", name="rcp")
                        nc.vector.reciprocal(rcp[:], psy[64:65, :])
                        rb = smalle.tile([64, 512], F32, tag="rb", name="rb")
                        nc.gpsimd.partition_broadcast(rb[:], rcp[:])
                        nc.vector.tensor_mul(
                            y_stack[pl][po:po + 64, q0:q0 + 512],
                            psy[0:64, :], rb[:],
                        )

        # ---- o-proj ----
        with tc.tile_pool(name="opsum", bufs=2, space="PSUM") as opsum, \
             tc.tile_pool(name="outp", bufs=2) as outp:
            for tt in range(NT):
                ot = outp.tile([128, C], F32, tag="ot", name="ot")
                for cg in range(2):
                    pso = opsum.tile([128, 512], F32, tag="pso", name="pso")
                    for pl in range(4):
                        nc.tensor.matmul(
                            pso[:],
                            r(y_stack[pl][:, tt * 128:(tt + 1) * 128]),
                            r(wo_sb[:, pl, cg * 512:(cg + 1) * 512]),
                            start=(pl == 0), stop=(pl == 3),
                        )
                    nc.any.tensor_copy(ot[:, cg * 512:(cg + 1) * 512], pso[:])
                nc.sync.dma_start(
                    out_d[tt * 128:(tt + 1) * 128, :], ot[:]
                )

    nc.compile()
    return nc


def host_prep(freqs, q_scale, k_scale):
    """Build rope constant tensors (shared across cores)."""
    c = np.cos(freqs[:, 0::2]).astype(np.float32)   # (T, 16)
    s = np.sin(freqs[:, 0::2]).astype(np.float32)
    consts = {}
    for nm, scale in (("q", q_scale), ("k", k_scale), ("v", np.ones(D, np.float32))):
        scale = np.asarray(scale, np.float32)
        cf = np.empty((T, D), np.float32)
        cf[:, 0:L:2] = c * scale[0:L:2][None, :]
        cf[:, 1:L:2] = c * scale[1:L:2][None, :]
        cf[:, L:] = scale[L:][None, :]
        se = (s * scale[1:L:2][None, :]).astype(np.float32)   # mult q_odd -> even
        so = (s * scale[0:L:2][None, :]).astype(np.float32)   # mult q_even -> odd
        consts[f"cf{nm}"] = np.ascontiguousarray(cf)
        consts[f"se{nm}"] = np.ascontiguousarray(se)
        consts[f"so{nm}"] = np.ascontiguousarray(so)
    consts["identf"] = np.eye(128, dtype=np.float32)
    ii = np.arange(128)
    consts["tri"] = (ii[:, None] <= ii[None, :]).astype(np.float32)
    return consts


_NC_CACHE = {}


def get_nc():
    if "nc" not in _NC_CACHE:
        _NC_CACHE["nc"] = build_program()
    return _NC_CACHE["nc"]


def make_in_maps(x, encoded_data, freqs, attn_bias, Wq, Wk, Wv, Wo,
                 q_scale, k_scale):
    consts = host_prep(np.asarray(freqs, np.float32),
                       np.asarray(q_scale, np.float32),
                       np.asarray(k_scale, np.float32))
    x = np.asarray(x, np.float32)
    e = np.asarray(encoded_data, np.float32)
    ab = np.asarray(attn_bias, np.float32)
    Wq = np.asarray(Wq, np.float32)
    Wk = np.ascontiguousarray(np.asarray(Wk, np.float32))
    Wv = np.ascontiguousarray(np.asarray(Wv, np.float32))
    Wo = np.asarray(Wo, np.float32)
    in_maps = []
    for core in range(8):
        b, g = core // 2, core % 2
        m = dict(consts)
        m["xb"] = np.ascontiguousarray(x[b])
        m["eb"] = np.ascontiguousarray(e[b])
        m["wq"] = np.ascontiguousarray(Wq[:, g * 512:(g + 1) * 512])
        m["wk"] = Wk
        m["wv"] = Wv
        m["wo"] = np.ascontiguousarray(Wo[g * 512:(g + 1) * 512, :])
        m["bias"] = np.ascontiguousarray(ab[g * HG:(g + 1) * HG])
        in_maps.append(m)
    return in_maps


def kernel(x, encoded_data, freqs, attn_bias, Wq, Wk, Wv, Wo,
           q_scale, k_scale):
    nc = get_nc()
    in_maps = make_in_maps(x, encoded_data, freqs, attn_bias,
                           Wq, Wk, Wv, Wo, q_scale, k_scale)
    res = run_bass_kernel_spmd(nc, in_maps, core_ids=list(range(8)))
    out = np.empty((B, T, C), np.float32)
    for b in range(B):
        out[b] = res.results[2 * b]["out"] + res.results[2 * b + 1]["out"]
    return out
